# revision 54
# baseline (speedup 1.0000x reference)
"""Causal multi-head attention (B=4, L=S=2048, H=16, E=D=128) on 8 trn2 cores.

Strategy:
  - Shard the 64 (batch, head) pairs across 8 cores, 8 pairs each.
  - Host pre-transposes Q and K to [E, L] / [E, S] (contraction dim on
    partitions) and pre-permutes V to [p, s_tile, d], all cast to bf16.
  - Per head, per 512-wide L chunk: PE computes causal score tiles
    ST[s_tile, l] = K^T.T @ Q^T, ACT applies exp(scale * x) (no max
    subtraction: scaled scores are ~N(0,1), bounded ~ +-7, so fp32 exp is
    exact enough and cannot overflow), GPSIMD zero-fills the
    upper-triangular part of diagonal tiles, then PE accumulates
    O^T[d, l] += V.T @ EST and rowsum[l] += ones.T @ EST.
  - Unnormalized O^T [d, l] and rowsums ship back as fp32; the host divides
    and transposes back into (B, L, H, D).
"""

import sys

if "/opt/trn_rl_repo" not in sys.path:
    sys.path.insert(0, "/opt/trn_rl_repo")

import numpy as np
import ml_dtypes

B, L, H, E = 4, 2048, 16, 128
S, D = L, E
N_CORES = 8
HEADS_PER_CORE = (B * H) // N_CORES
SCALE = 1.0 / float(np.sqrt(E))
EXP_SHIFT = -2.0  # exp(scale*x + EXP_SHIFT); cancels in the normalization
P = 128
LCHUNK = 512

_CACHE = {}


def _build(heads, seq):
    """Build the Bass module for `heads` (b,h) pairs of seq-length `seq`."""
    import concourse.tile as tile
    from concourse import bacc, mybir
    from contextlib import ExitStack

    n_stiles = seq // P
    n_chunks = seq // LCHUNK
    stiles_per_chunk = n_stiles // n_chunks  # s-tiles that fit one l-chunk

    bf16 = mybir.dt.bfloat16
    f32 = mybir.dt.float32

    nc = bacc.Bacc("TRN2", target_bir_lowering=False, debug=False)
    qt = nc.dram_tensor("qt", [heads, P, seq], bf16, kind="ExternalInput").ap()
    kt = nc.dram_tensor("kt", [heads, P, seq], bf16, kind="ExternalInput").ap()
    vp = nc.dram_tensor("vp", [heads, P, n_stiles, P], bf16, kind="ExternalInput").ap()
    # masks[j][p, f] = 1.0 if f >= j*128 + p else 0 (triangular keep-mask for
    # the s-tile whose diagonal sits j*128 columns into the l-chunk)
    mk = nc.dram_tensor(
        "mk", [P, stiles_per_chunk, LCHUNK], bf16, kind="ExternalInput"
    ).ap()
    ot = nc.dram_tensor("ot", [heads, P, seq], f32, kind="ExternalOutput").ap()
    osum = nc.dram_tensor("osum", [heads, seq], f32, kind="ExternalOutput").ap()

    with tile.TileContext(nc) as tc, ExitStack() as ctx:
        const = ctx.enter_context(tc.tile_pool(name="const", bufs=1))
        inpool = ctx.enter_context(tc.tile_pool(name="inp", bufs=2))
        # separate pools so unmasked est slots carry PE-only deps (fewer
        # event-semaphore splits on the Scalar queue)
        est_pool = ctx.enter_context(tc.tile_pool(name="est", bufs=4))
        estm_pool = ctx.enter_context(tc.tile_pool(name="estm", bufs=4))
        part_pool = ctx.enter_context(tc.tile_pool(name="part", bufs=4))
        out_pool = ctx.enter_context(tc.tile_pool(name="out", bufs=2))
        sums_pool = ctx.enter_context(tc.tile_pool(name="sums", bufs=2))
        st_psum = ctx.enter_context(tc.tile_pool(name="stp", bufs=2, space="PSUM"))
        ot_psum = ctx.enter_context(tc.tile_pool(name="otp", bufs=2, space="PSUM"))

        ones = const.tile([P, 1], bf16)
        nc.gpsimd.memset(ones[:], 1.0)
        nbias = const.tile([P, 1], f32)
        nc.gpsimd.memset(nbias[:], float(EXP_SHIFT))
        masks = const.tile([P, stiles_per_chunk, LCHUNK], bf16)
        nc.sync.dma_start(masks[:], mk)

        for h in range(heads):
            ktile = inpool.tile([P, seq], bf16, tag="kt")
            nc.sync.dma_start(ktile[:], kt[h])
            qtile = inpool.tile([P, seq], bf16, tag="qt")
            nc.sync.dma_start(qtile[:], qt[h])
            vtile = inpool.tile([P, n_stiles, P], bf16, tag="v")
            nc.sync.dma_start(vtile[:], vp[h])

            for c in range(n_chunks):
                n_st = stiles_per_chunk * (c + 1)  # causal s-tiles for chunk
                # one 2-bank tile: bank 0 = O^T accum, bank 1 row 0 = rowsums
                oacc = ot_psum.tile([P, 2, LCHUNK], f32)
                otp = oacc[:, 0, :]
                sump = oacc[0:1, 1, :]
                l_lo = c * LCHUNK

                for pair in range(n_st // 2):
                    s0 = 2 * pair
                    masked = (s0 + 1) * P + P > l_lo
                    stp = st_psum.tile([P, 2, LCHUNK], f32)
                    for i in range(2):
                        s = s0 + i
                        nc.tensor.matmul(
                            stp[:, i, :],
                            lhsT=ktile[:, s * P : (s + 1) * P],
                            rhs=qtile[:, l_lo : l_lo + LCHUNK],
                            start=True,
                            stop=True,
                        )
                    pool = estm_pool if masked else est_pool
                    est = pool.tile([P, 2, LCHUNK], bf16)
                    # exp(scale*x - 2): the -2 cancels in normalization and
                    # keeps exp outputs < 50 (fp8e4m3 saturates at 448)
                    nc.scalar.activation(
                        est[:],
                        stp[:],
                        mybir.ActivationFunctionType.Exp,
                        bias=nbias[:],
                        scale=SCALE,
                    )
                    for i in range(2):
                        s = s0 + i
                        if s * P + P > l_lo:  # tile touches the diagonal
                            # mask on DVE (GPSIMD's ~1us queue latency
                            # stalls the chunk-end AV matmuls)
                            j = s - c * stiles_per_chunk
                            nc.vector.tensor_mul(
                                est[:, i, :], est[:, i, :], masks[:, j, :]
                            )
                    for i in range(2):
                        s = s0 + i
                        nc.tensor.matmul(
                            otp[:],
                            lhsT=vtile[:, s, :],
                            rhs=est[:, i, :],
                            start=s == 0,
                            stop=s == n_st - 1,
                        )
                    # rowsum: pair-add on DVE, then one ones-matmul per pair
                    partial = part_pool.tile([P, LCHUNK], bf16)
                    nc.vector.tensor_add(partial[:], est[:, 0, :], est[:, 1, :])
                    nc.tensor.matmul(
                        sump[:],
                        lhsT=ones[:],
                        rhs=partial[:],
                        start=pair == 0,
                        stop=pair == n_st // 2 - 1,
                    )

                osb = out_pool.tile([P, LCHUNK], f32)
                nc.vector.tensor_copy(osb[:], otp[:])
                ssb = sums_pool.tile([1, LCHUNK], f32)
                nc.vector.tensor_copy(ssb[:], sump[:])
                nc.sync.dma_start(ot[h][:, l_lo : l_lo + LCHUNK], osb[:])
                nc.sync.dma_start(osum[h][None, l_lo : l_lo + LCHUNK], ssb[:])

    nc.compile()
    return nc


def _get_nc(heads, seq):
    key = (heads, seq)
    if key not in _CACHE:
        _CACHE[key] = _build(heads, seq)
    return _CACHE[key]


def _prep_inputs(queries, keys, values):
    """Host-side shard + layout prep. Returns per-core input maps."""
    bf16 = ml_dtypes.bfloat16
    q = np.asarray(queries, dtype=np.float32)
    k = np.asarray(keys, dtype=np.float32)
    v = np.asarray(values, dtype=np.float32)
    b, l, h, e = q.shape
    s = k.shape[1]
    n_stiles = s // P

    # [B,L,H,E] -> [B*H, E, L] (pair index = b*H + h)
    qt = np.ascontiguousarray(q.transpose(0, 2, 3, 1).reshape(b * h, e, l)).astype(bf16)
    kt = np.ascontiguousarray(k.transpose(0, 2, 3, 1).reshape(b * h, e, s)).astype(bf16)
    # [B,S,H,D] -> [B*H, S, D] -> [B*H, P, n_stiles, D] with vp[p, st, d] = V[st*P+p, d]
    vv = v.transpose(0, 2, 1, 3).reshape(b * h, n_stiles, P, v.shape[3])
    vp = np.ascontiguousarray(vv.transpose(0, 2, 1, 3)).astype(bf16)

    # diagonal masks: mk[p, j, f] = 1.0 if f >= j*P + p else 0
    spc = LCHUNK // P
    pp = np.arange(P)[:, None, None]
    jj = np.arange(spc)[None, :, None]
    ff = np.arange(LCHUNK)[None, None, :]
    mk = (ff >= jj * P + pp).astype(bf16)

    hpc = (b * h) // N_CORES
    in_maps = []
    for ci in range(N_CORES):
        sl = slice(ci * hpc, (ci + 1) * hpc)
        in_maps.append({"qt": qt[sl], "kt": kt[sl], "vp": vp[sl], "mk": mk})
    return in_maps


def _assemble_output(results, b, l, h, d):
    """Per-core ot [hpc, D, L] (unnormalized) + osum [hpc, L] -> (B, L, H, D)."""
    ot_all = np.concatenate([r["ot"] for r in results], axis=0)  # [B*H, D, L]
    sums = np.concatenate([r["osum"] for r in results], axis=0)  # [B*H, L]
    ot_all = ot_all / sums[:, None, :]
    out = ot_all.transpose(0, 2, 1).reshape(b, h, l, d).transpose(0, 2, 1, 3)
    return np.ascontiguousarray(out, dtype=np.float32)


def kernel(queries, keys, values):
    from concourse.bass_utils import run_bass_kernel_spmd

    q = np.asarray(queries)
    b, l, h, e = q.shape
    nc = _get_nc((b * h) // N_CORES, l)
    in_maps = _prep_inputs(queries, keys, values)
    res = run_bass_kernel_spmd(nc, in_maps, list(range(N_CORES)))
    return _assemble_output(res.results, b, l, h, values.shape[3])
